# revision 27
# baseline (speedup 1.0000x reference)
"""Trainium2 Bass kernel for nn_Attn_30623116820602.

Low-rank-projected causal multi-head attention:
  q/k/v = (x @ A) @ B  (rank 192), RoPE on q,k, causal softmax attention,
  output projection.  x: [128, 256, 768] fp32.

Sharding: pure data-parallel over batch (16 items per core, 8 cores).
Feature-major layout (d_model on partitions) throughout; host pre/post
transposes.  All matmul inputs are bf16 (PSUM accumulates fp32).

Structure (per pair of batch items = 512 token columns):
  - proj1 packs the 3 rank-192 outputs into 5 (not 6) 128-row tiles:
    [q0:128 | q128:192+k0:64 | k64:192 | v0:128 | v128:192+pad].
  - RoPE rotate-half is two partition-shifted SBUF-to-SBUF DMA copies
    of the bf16 head tile (the sign flip is folded into the sin table),
    so the rotation costs no tensor-engine cycles at all.
  - Causal block structure is exploited: the fully-masked
    (keytile1 x querytile0) block is never computed -- not in scores,
    exp, denominators, nor the AV matmul.  E layout per (item, head) is
    [kt0q0 | kt1q1 | kt0q1] so the two triangular diagonal blocks are
    adjacent and share one mask multiply.
  - Softmax denominators: matmuls against an all-ones [128,128]
    stationary produce the denominator already replicated on all 128
    partitions (same column cost as a ones-vector), so the reciprocal
    runs as an efficient full-width [128,512] vector op and no
    partition broadcast is needed.  No DRAM round trip (the fp32
    baseline's 4-hop DRAM chain serialized the pipeline at ~16us/pair).
  - x loads are prefetched one pair ahead on the gpsimd DMA queue;
    output stores ride the sync queue; the output projection of pair
    N-1 is interleaved into pair N's attention to keep the PE busy.
"""

import math
import sys

sys.path.insert(0, "/opt/trn_rl_repo")

import numpy as np
import ml_dtypes


def _to_bf16(a):
    return a.astype(ml_dtypes.bfloat16)


B, T, D = 128, 256, 768
H, HD = 6, 128
RANK = 192
N_CORES = 8
B_LOC = B // N_CORES  # 16
N_PAIRS = B_LOC // 2  # 8 (2 batch items per pipeline iteration)
SCALE = 1.0 / math.sqrt(HD)

_CACHE = {}


def build_program(n_pairs=N_PAIRS):
    import concourse.tile as tile
    from concourse import bacc, mybir
    from contextlib import ExitStack

    f32 = mybir.dt.float32
    bf16 = mybir.dt.bfloat16
    TOK = n_pairs * 512

    nc = bacc.Bacc("TRN2", target_bir_lowering=False, debug=False,
                   num_devices=N_CORES)

    def din(name, shape):
        return nc.dram_tensor(name, shape, bf16, kind="ExternalInput").ap()

    xT = din("xT", [6, 128, TOK])
    Ap_l = din("Ap_l", [6, 128, 640])
    qBp_l = din("qBp_l", [2, 128, 768])
    kBp_l = din("kBp_l", [2, 128, 768])
    vBp_l = din("vBp_l", [2, 128, 768])
    ow_l = din("ow_l", [6, 128, 768])
    cosT = din("cosT", [128, 256])
    sinT = din("sinT", [128, 256])
    mask_l = din("mask_l", [128, 384])
    ones_l = din("ones_l", [128, 128])
    outT = nc.dram_tensor("outT", [6, 128, TOK], f32, kind="ExternalOutput").ap()

    with tile.TileContext(nc) as tc:
        with ExitStack() as ctx:
            wp = ctx.enter_context(tc.tile_pool(name="w", bufs=1))
            xp = ctx.enter_context(tc.tile_pool(name="xt", bufs=2))
            xrp = ctx.enter_context(tc.tile_pool(name="xr", bufs=2))
            rawp = ctx.enter_context(tc.tile_pool(name="raw", bufs=2))
            qkp = ctx.enter_context(tc.tile_pool(name="qk", bufs=1))
            vp_ = ctx.enter_context(tc.tile_pool(name="vsb", bufs=2))
            tp = ctx.enter_context(tc.tile_pool(name="tmp", bufs=2))
            ep = ctx.enter_context(tc.tile_pool(name="eexp", bufs=8))
            dp = ctx.enter_context(tc.tile_pool(name="den", bufs=2))
            bp = ctx.enter_context(tc.tile_pool(name="bcast", bufs=2))
            orp = ctx.enter_context(tc.tile_pool(name="oraw", bufs=2))
            aop = ctx.enter_context(tc.tile_pool(name="ao", bufs=2))
            fp = ctx.enter_context(tc.tile_pool(name="fout", bufs=2))
            ps = ctx.enter_context(tc.tile_pool(name="ps", bufs=3, space="PSUM"))
            pm = ctx.enter_context(tc.tile_pool(name="pm", bufs=2, space="PSUM"))
            spp = ctx.enter_context(tc.tile_pool(name="sp", bufs=3, space="PSUM"))

            def psum():
                return ps.tile([128, 512], f32, tag="ps", name="psb")

            # ---- resident weights / constants (gpsimd DMA queue) ----
            def wload(name, src, shape, perm=None):
                t = wp.tile(shape, bf16, tag=name, name=name)
                nc.gpsimd.dma_start(t[:], src.rearrange(perm) if perm else src)
                return t

            # order matters: pair 0 needs A/qBp/kBp/cos/sin first; the
            # output projection weights are not needed until pair 1.
            A_s = wload("Ap", Ap_l, [128, 6, 640], "k p m -> p k m")
            qBp_s = wload("qBp", qBp_l, [128, 2, 768], "k p m -> p k m")
            kBp_s = wload("kBp", kBp_l, [128, 2, 768], "k p m -> p k m")
            cos_s = wload("cos", cosT, [128, 256])
            sin_s = wload("sin", sinT, [128, 256])
            vBp_s = wload("vBp", vBp_l, [128, 2, 768], "k p m -> p k m")
            mask_s = wload("mask", mask_l, [128, 384])
            ones_s = wload("ones", ones_l, [128, 128])
            ow_s = wload("ow", ow_l, [128, 6, 768], "k p m -> p k m")

            def emit_outproj(aosb_prev, pr_prev, mts, half=None):
                w = 512 if half is None else 256
                c0 = 0 if half in (None, 0) else 256
                tokp = slice(pr_prev * 512 + c0, pr_prev * 512 + c0 + w)
                for mt in mts:
                    fps = psum()
                    for kt in range(6):
                        nc.tensor.matmul(
                            fps[:, 0:w],
                            ow_s[:, kt, mt * 128:(mt + 1) * 128],
                            aosb_prev[:, kt, c0:c0 + w],
                            start=(kt == 0), stop=(kt == 5))
                    fout = fp.tile([128, 512], f32, tag="fout", name="fout")
                    nc.scalar.copy(fout[:, 0:w], fps[:, 0:w])
                    nc.sync.dma_start(outT[mt, :, tokp], fout[:, 0:w])

            # prefetch first x pair
            xts = [None] * n_pairs

            def load_xt(p):
                t = xp.tile([128, 6, 512], bf16, tag="xt", name="xt")
                nc.gpsimd.dma_start(
                    t[:], xT[:, :, p * 512:(p + 1) * 512].rearrange("k p t -> p k t"))
                xts[p] = t

            load_xt(0)

            prev = None
            for prx in range(n_pairs):
                if prx + 1 < n_pairs:
                    load_xt(prx + 1)
                xt = xts[prx]

                # ---- proj1: packed rank tiles [q|q+k|k|v|v] ----
                xr = xrp.tile([128, 5, 512], bf16, tag="xr", name="xr")
                for rt in range(5):
                    mm = psum()
                    for kt in range(6):
                        nc.tensor.matmul(
                            mm[:],
                            A_s[:, kt, rt * 128:(rt + 1) * 128],
                            xt[:, kt, :],
                            start=(kt == 0), stop=(kt == 5))
                    nc.scalar.copy(xr[:, rt, :], mm[:])

                # ---- proj2 + RoPE for q and k (feature-major) ----
                # q contracts xr tiles {0,1}; k contracts {1,2} (B rows
                # zero-padded on host where tiles are shared).
                qsb = qkp.tile([128, 6, 512], bf16, tag="qsb", name="qsb")
                ksb = qkp.tile([128, 6, 512], bf16, tag="ksb", name="ksb")
                # pass 1: all head matmuls + PSUM->SBUF copies into one
                # per-projection raw tile
                raws = {}
                for pname in ("q", "k"):
                    raws[pname] = (
                        rawp.tile([128, 6, 512], bf16, tag=f"raw{pname}",
                                  name=f"raw{pname}"),
                        rawp.tile([128, 6, 512], bf16, tag=f"rot{pname}",
                                  name=f"rot{pname}"))
                for h in range(6):
                    hc = slice(h * 128, (h + 1) * 128)
                    for pname, B_s, t0 in (("q", qBp_s, 0), ("k", kBp_s, 1)):
                        p_main = pm.tile([128, 512], f32, tag="pm", name="pm")
                        for kt in range(2):
                            nc.tensor.matmul(
                                p_main[:], B_s[:, kt, hc], xr[:, t0 + kt, :],
                                start=(kt == 0), stop=(kt == 1))
                        nc.scalar.copy(raws[pname][0][:, h, :], p_main[:])
                # rotate-half: one partition-shift DMA pair per projection
                # (sign pattern is folded into the sin table)
                for pname in ("q", "k"):
                    raw, rot = raws[pname]
                    nc.sync.dma_start(rot[0:64], raw[64:128])
                    nc.sync.dma_start(rot[64:128], raw[0:64])
                # pass 2: RoPE combine, all-bf16 vector ops back-to-back
                for pname, sb in (("q", qsb), ("k", ksb)):
                    raw, rot = raws[pname]
                    for h in range(6):
                        tmp = tp.tile([128, 512], bf16, tag="ropetmp",
                                      name="ropetmp")
                        nc.vector.tensor_tensor(
                            sb[:, h, :].rearrange("p (b q) -> p b q", b=2),
                            raw[:, h, :].rearrange("p (b q) -> p b q", b=2),
                            cos_s[:, None, :].to_broadcast((128, 2, 256)),
                            mybir.AluOpType.mult)
                        nc.vector.tensor_tensor(
                            tmp[:].rearrange("p (b q) -> p b q", b=2),
                            rot[:, h, :].rearrange("p (b q) -> p b q", b=2),
                            sin_s[:, None, :].to_broadcast((128, 2, 256)),
                            mybir.AluOpType.mult)
                        nc.vector.tensor_tensor(
                            sb[:, h, :], sb[:, h, :], tmp[:],
                            mybir.AluOpType.add)

                # ---- proj2 for v (token-major), contracts xr tiles {3,4} ----
                vsb = vp_.tile([128, 4, 768], bf16, tag="vsb", name="vsb")
                for mt in range(4):
                    for nch in range(2):
                        vps = psum()
                        for kt in range(2):
                            nc.tensor.matmul(
                                vps[:, 0:384],
                                xr[:, 3 + kt, mt * 128:(mt + 1) * 128],
                                vBp_s[:, kt, nch * 384:(nch + 1) * 384],
                                start=(kt == 0), stop=(kt == 1))
                        nc.scalar.copy(vsb[:, mt, nch * 384:(nch + 1) * 384],
                                       vps[:, 0:384])

                # ---- attention (per batch item) ----
                # E layout per (b, h): [kt0q0 | kt0q1 | kt1q1], each 128 wide.
                aosb = aop.tile([128, 6, 512], bf16, tag="aosb", name="aosb")
                for b in range(2):
                    if prev is not None:
                        emit_outproj(prev[0], prev[1], range(3 * b, 3 * b + 3))
                    i_bD = bp.tile([128, 1536], f32, tag="ibD", name="ibD")
                    Es = []
                    for h in range(6):
                        base = h * 512 + 0  # qsb is [128, 6, 512]
                        qh = qsb[:, h, b * 256:(b + 1) * 256]
                        kh = ksb[:, h, b * 256:(b + 1) * 256]
                        sp = spp.tile([128, 512], f32, tag="sp", name="sp")
                        nc.tensor.matmul(sp[:, 0:256], kh[:, 0:128],
                                         qh[:, 0:256], start=True, stop=True)
                        nc.tensor.matmul(sp[:, 256:384], kh[:, 128:256],
                                         qh[:, 128:256], start=True, stop=True)
                        E = ep.tile([128, 384], bf16, tag="E", name="E")
                        nc.scalar.activation(
                            E[:], sp[:, 0:384],
                            mybir.ActivationFunctionType.Exp, scale=SCALE)
                        # one mask multiply: [tril | ones | tril]
                        nc.vector.tensor_tensor(
                            E[:], E[:], mask_s[:],
                            mybir.AluOpType.mult)
                        Es.append(E)
                    # denominators: all-ones stationary replicates the key-sum
                    # across all 128 partitions (broadcast comes for free)
                    dslots = []
                    for h in range(6):
                        E = Es[h]
                        s, c = h // 2, (h % 2) * 256
                        if h % 2 == 0:
                            dslots.append(psum())
                        dps = dslots[s]
                        nc.tensor.matmul(dps[:, c:c + 256], ones_s[:],
                                         E[:, 0:256], start=True, stop=False,
                                         skip_group_check=True)
                        nc.tensor.matmul(dps[:, c + 128:c + 256], ones_s[:],
                                         E[:, 256:384], start=False, stop=True,
                                         skip_group_check=True)
                        if h % 2 == 1:
                            sc = slice(s * 512, (s + 1) * 512)
                            nc.vector.reciprocal_approx_fast(
                                i_bD[:, sc], dps[:, :])
                    # attention output (feature-major), normalized straight
                    # out of PSUM
                    for h in range(6):
                        E = Es[h]
                        o2 = psum()
                        v0 = vsb[:, b * 2 + 0, h * 128:(h + 1) * 128]
                        v1 = vsb[:, b * 2 + 1, h * 128:(h + 1) * 128]
                        nc.tensor.matmul(o2[:, 0:256], v0, E[:, 0:256],
                                         start=True, stop=False,
                                         skip_group_check=True)
                        nc.tensor.matmul(o2[:, 128:256], v1, E[:, 256:384],
                                         start=False, stop=True,
                                         skip_group_check=True)
                        nc.vector.tensor_tensor(
                            aosb[:, h, b * 256:(b + 1) * 256],
                            o2[:, 0:256],
                            i_bD[:, h * 256:(h + 1) * 256],
                            mybir.AluOpType.mult)
                    if prx == n_pairs - 1:
                        emit_outproj(aosb, prx, range(6), half=b)
                prev = (aosb, prx)

    nc.compile()
    return nc


def _rope_tables():
    inv = 1.0 / (10000.0 ** (np.arange(0, HD, 2, dtype=np.float32) / HD))
    t = np.arange(T, dtype=np.float32)
    freqs = np.outer(t, inv)                      # [T, 64]
    emb = np.concatenate([freqs, freqs], axis=-1)  # [T, 128]
    return np.cos(emb).astype(np.float32), np.sin(emb).astype(np.float32)


def _prep_shared(qA, qB, kA, kB, vA, vB, o_w):
    """Host-side weight/constant layouts (shared by all cores)."""
    def a_r(A):  # [768,192] -> [6,128,192]
        return A.reshape(6, 128, RANK)

    qA_r, kA_r, vA_r = a_r(qA), a_r(kA), a_r(vA)
    Ap = np.zeros((6, 128, 640), np.float32)
    Ap[:, :, 0:128] = qA_r[:, :, 0:128]
    Ap[:, :, 128:192] = qA_r[:, :, 128:192]
    Ap[:, :, 192:256] = kA_r[:, :, 0:64]
    Ap[:, :, 256:384] = kA_r[:, :, 64:192]
    Ap[:, :, 384:512] = vA_r[:, :, 0:128]
    Ap[:, :, 512:576] = vA_r[:, :, 128:192]

    qBp = np.zeros((2, 128, D), np.float32)
    qBp[0] = qB[0:128]
    qBp[1, 0:64] = qB[128:192]

    kBp = np.zeros((2, 128, D), np.float32)
    kBp[0, 64:128] = kB[0:64]
    kBp[1] = kB[64:192]

    vBp = np.zeros((2, 128, D), np.float32)
    vBp[0] = vB[0:128]
    vBp[1, 0:64] = vB[128:192]

    cos, sin = _rope_tables()
    cosT = np.ascontiguousarray(cos.T)  # [128, 256]
    # fold the rotate-half sign pattern into the sin table: the device's
    # rot tile holds q[(f+64) % 128], which needs -sin for f<64, +sin after
    sinT = np.ascontiguousarray(sin.T).copy()
    sinT[0:64] *= -1.0

    p = np.arange(128)[:, None]
    j = np.arange(128)[None, :]
    tril = (p <= j).astype(np.float32)  # keys (partitions) <= queries (cols)
    mask = np.concatenate(
        [tril, np.ones((128, 128), np.float32), tril], axis=1)  # [128, 384]

    return {
        "Ap_l": _to_bf16(np.ascontiguousarray(Ap)),
        "qBp_l": _to_bf16(qBp), "kBp_l": _to_bf16(kBp), "vBp_l": _to_bf16(vBp),
        "ow_l": _to_bf16(np.ascontiguousarray(o_w.reshape(6, 128, D))),
        "cosT": _to_bf16(cosT), "sinT": _to_bf16(sinT),
        "mask_l": _to_bf16(mask),
        "ones_l": _to_bf16(np.ones((128, 128), np.float32)),
    }


def x_to_xT(xc):
    """[b, T, D] -> [6, 128, b*T] feature-major, batch-major tokens."""
    nb = xc.shape[0]
    return np.ascontiguousarray(
        _to_bf16(xc).reshape(nb, T, 6, 128).transpose(2, 3, 0, 1).reshape(6, 128, nb * T))


def outT_to_out(oT, nb):
    return np.ascontiguousarray(
        oT.reshape(6, 128, nb, T).transpose(2, 3, 0, 1).reshape(nb, T, D))


def kernel(x, qA, qB, kA, kB, vA, vB, o_w):
    from concourse import bass_utils

    if "nc" not in _CACHE:
        _CACHE["nc"] = build_program(N_PAIRS)
    nc = _CACHE["nc"]

    shared = _prep_shared(
        np.asarray(qA, np.float32), np.asarray(qB, np.float32),
        np.asarray(kA, np.float32), np.asarray(kB, np.float32),
        np.asarray(vA, np.float32), np.asarray(vB, np.float32),
        np.asarray(o_w, np.float32))
    x = np.asarray(x, np.float32)

    in_maps = []
    for c in range(N_CORES):
        m = dict(shared)
        m["xT"] = x_to_xT(x[c * B_LOC:(c + 1) * B_LOC])
        in_maps.append(m)

    res = bass_utils.run_bass_kernel_spmd(
        nc, in_maps, core_ids=list(range(N_CORES)))
    out = np.empty((B, T, D), np.float32)
    for c in range(N_CORES):
        out[c * B_LOC:(c + 1) * B_LOC] = outT_to_out(
            res.results[c]["outT"], B_LOC)
    return out


# revision 34
# speedup vs baseline: 1.1665x; 1.1665x over previous
"""Trainium2 Bass kernel for nn_Attn_30623116820602.

Low-rank-projected causal multi-head attention:
  q/k/v = (x @ A) @ B  (rank 192), RoPE on q,k, causal softmax attention,
  output projection.  x: [128, 256, 768] fp32.

Sharding: pure data-parallel over batch (16 items per core, 8 cores).
Feature-major layout (d_model on partitions) throughout; host pre/post
transposes.  All matmul inputs are bf16 (PSUM accumulates fp32).

Structure (per pair of batch items = 512 token columns):
  - proj1 packs the 3 rank-192 outputs into 5 (not 6) 128-row tiles:
    [q0:128 | q128:192+k0:64 | k64:192 | v0:128 | v128:192+pad].
  - RoPE rotate-half comes from one extra matmul with a shared 128x128
    +-1 permutation matrix P (contraction 128) instead of duplicated
    rank-contraction weight matmuls.  (Partition-shift DMA variants
    measured slower: SBUF-to-SBUF DMA latency stalls the vector queue.)
  - Causal block structure is exploited: the fully-masked
    (keytile1 x querytile0) block is never computed -- not in scores,
    exp, denominators, nor the AV matmul.  E layout per (item, head) is
    [kt0q0 | kt1q1 | kt0q1] so the two triangular diagonal blocks are
    adjacent and share one mask multiply.
  - Softmax denominators: matmuls against an all-ones [128,128]
    stationary produce the denominator already replicated on all 128
    partitions (same column cost as a ones-vector), so the reciprocal
    runs as an efficient full-width [128,512] vector op and no
    partition broadcast is needed.  No DRAM round trip (the fp32
    baseline's 4-hop DRAM chain serialized the pipeline at ~16us/pair).
  - x loads are prefetched one pair ahead on the gpsimd DMA queue;
    output stores ride the sync queue; the output projection of pair
    N-1 is interleaved into pair N's attention to keep the PE busy.
"""

import math
import sys

sys.path.insert(0, "/opt/trn_rl_repo")

import numpy as np
import ml_dtypes


def _to_bf16(a):
    return a.astype(ml_dtypes.bfloat16)


B, T, D = 128, 256, 768
H, HD = 6, 128
RANK = 192
N_CORES = 8
B_LOC = B // N_CORES  # 16
N_PAIRS = B_LOC // 2  # 8 (2 batch items per pipeline iteration)
SCALE = 1.0 / math.sqrt(HD)

_CACHE = {}


def build_program(n_pairs=N_PAIRS):
    import concourse.tile as tile
    from concourse import bacc, mybir
    from contextlib import ExitStack

    f32 = mybir.dt.float32
    bf16 = mybir.dt.bfloat16
    TOK = n_pairs * 512

    nc = bacc.Bacc("TRN2", target_bir_lowering=False, debug=False,
                   num_devices=N_CORES)

    def din(name, shape):
        return nc.dram_tensor(name, shape, bf16, kind="ExternalInput").ap()

    xT = din("xT", [6, 128, TOK])
    Ap_l = din("Ap_l", [6, 128, 640])
    qBp_l = din("qBp_l", [2, 128, 768])
    kBp_l = din("kBp_l", [2, 128, 768])
    vBp_l = din("vBp_l", [2, 128, 768])
    ow_l = din("ow_l", [6, 128, 768])
    P_l = din("P_l", [128, 128])
    cosT = din("cosT", [128, 256])
    sinT = din("sinT", [128, 256])
    mask_l = din("mask_l", [128, 384])
    ones_l = din("ones_l", [128, 128])
    outT = nc.dram_tensor("outT", [6, 128, TOK], f32, kind="ExternalOutput").ap()

    with tile.TileContext(nc) as tc:
        with ExitStack() as ctx:
            wp = ctx.enter_context(tc.tile_pool(name="w", bufs=1))
            xp = ctx.enter_context(tc.tile_pool(name="xt", bufs=2))
            xrp = ctx.enter_context(tc.tile_pool(name="xr", bufs=2))
            rawp = ctx.enter_context(tc.tile_pool(name="raw", bufs=2))
            qkp = ctx.enter_context(tc.tile_pool(name="qk", bufs=1))
            vp_ = ctx.enter_context(tc.tile_pool(name="vsb", bufs=2))
            tp = ctx.enter_context(tc.tile_pool(name="tmp", bufs=2))
            ep = ctx.enter_context(tc.tile_pool(name="eexp", bufs=8))
            dp = ctx.enter_context(tc.tile_pool(name="den", bufs=2))
            bp = ctx.enter_context(tc.tile_pool(name="bcast", bufs=2))
            orp = ctx.enter_context(tc.tile_pool(name="oraw", bufs=2))
            aop = ctx.enter_context(tc.tile_pool(name="ao", bufs=2))
            fp = ctx.enter_context(tc.tile_pool(name="fout", bufs=2))
            ps = ctx.enter_context(tc.tile_pool(name="ps", bufs=3, space="PSUM"))
            pm = ctx.enter_context(tc.tile_pool(name="pm", bufs=2, space="PSUM"))
            pr = ctx.enter_context(tc.tile_pool(name="pr", bufs=1, space="PSUM"))
            spp = ctx.enter_context(tc.tile_pool(name="sp", bufs=2, space="PSUM"))

            def psum():
                return ps.tile([128, 512], f32, tag="ps", name="psb")

            # ---- resident weights / constants (gpsimd DMA queue) ----
            def wload(name, src, shape, perm=None):
                t = wp.tile(shape, bf16, tag=name, name=name)
                nc.gpsimd.dma_start(t[:], src.rearrange(perm) if perm else src)
                return t

            # order matters: pair 0 needs A/qBp/kBp/cos/sin first; the
            # output projection weights are not needed until pair 1.
            A_s = wload("Ap", Ap_l, [128, 6, 640], "k p m -> p k m")
            qBp_s = wload("qBp", qBp_l, [128, 2, 768], "k p m -> p k m")
            kBp_s = wload("kBp", kBp_l, [128, 2, 768], "k p m -> p k m")
            P_s = wload("P", P_l, [128, 128])
            cos_s = wload("cos", cosT, [128, 256])
            sin_s = wload("sin", sinT, [128, 256])
            vBp_s = wload("vBp", vBp_l, [128, 2, 768], "k p m -> p k m")
            mask_s = wload("mask", mask_l, [128, 384])
            ones_s = wload("ones", ones_l, [128, 128])
            ow_s = wload("ow", ow_l, [128, 6, 768], "k p m -> p k m")

            def emit_outproj(aosb_prev, pr_prev, mts, half=None):
                w = 512 if half is None else 256
                c0 = 0 if half in (None, 0) else 256
                tokp = slice(pr_prev * 512 + c0, pr_prev * 512 + c0 + w)
                for mt in mts:
                    fps = psum()
                    for kt in range(6):
                        nc.tensor.matmul(
                            fps[:, 0:w],
                            ow_s[:, kt, mt * 128:(mt + 1) * 128],
                            aosb_prev[:, kt, c0:c0 + w],
                            start=(kt == 0), stop=(kt == 5))
                    fout = fp.tile([128, 512], f32, tag="fout", name="fout")
                    nc.scalar.copy(fout[:, 0:w], fps[:, 0:w])
                    nc.sync.dma_start(outT[mt, :, tokp], fout[:, 0:w])

            # prefetch first x pair
            xts = [None] * n_pairs

            def load_xt(p):
                t = xp.tile([128, 6, 512], bf16, tag="xt", name="xt")
                nc.gpsimd.dma_start(
                    t[:], xT[:, :, p * 512:(p + 1) * 512].rearrange("k p t -> p k t"))
                xts[p] = t

            load_xt(0)

            prev = None
            for prx in range(n_pairs):
                if prx + 1 < n_pairs:
                    load_xt(prx + 1)
                xt = xts[prx]

                # ---- proj1: packed rank tiles [q|q+k|k|v|v] ----
                xr = xrp.tile([128, 5, 512], bf16, tag="xr", name="xr")
                for rt in range(5):
                    mm = psum()
                    for kt in range(6):
                        nc.tensor.matmul(
                            mm[:],
                            A_s[:, kt, rt * 128:(rt + 1) * 128],
                            xt[:, kt, :],
                            start=(kt == 0), stop=(kt == 5))
                    nc.scalar.copy(xr[:, rt, :], mm[:])

                # ---- proj2 + RoPE for q and k (feature-major) ----
                # q contracts xr tiles {0,1}; k contracts {1,2} (B rows
                # zero-padded on host where tiles are shared).
                qsb = qkp.tile([128, 6, 512], bf16, tag="qsb", name="qsb")
                ksb = qkp.tile([128, 6, 512], bf16, tag="ksb", name="ksb")
                for h in range(6):
                    hc = slice(h * 128, (h + 1) * 128)
                    mains = {}
                    for pname, B_s, t0, sb in (
                            ("q", qBp_s, 0, qsb), ("k", kBp_s, 1, ksb)):
                        p_main = pm.tile([128, 512], f32, tag="pm", name="pm")
                        for kt in range(2):
                            nc.tensor.matmul(
                                p_main[:], B_s[:, kt, hc], xr[:, t0 + kt, :],
                                start=(kt == 0), stop=(kt == 1))
                        raw = rawp.tile([128, 512], bf16, tag=f"raw{pname}",
                                        name=f"raw{pname}")
                        nc.scalar.copy(raw[:], p_main[:])
                        mains[pname] = (raw, sb)
                    for pname in ("q", "k"):
                        raw, sb = mains[pname]
                        p_rot = pr.tile([128, 512], f32, tag="pr", name="pr")
                        nc.tensor.matmul(p_rot[:], P_s[:], raw[:],
                                         start=True, stop=True)
                        tmp = tp.tile([128, 512], bf16, tag="ropetmp",
                                      name="ropetmp")
                        nc.vector.tensor_tensor(
                            sb[:, h, :].rearrange("p (b q) -> p b q", b=2),
                            raw[:].rearrange("p (b q) -> p b q", b=2),
                            cos_s[:, None, :].to_broadcast((128, 2, 256)),
                            mybir.AluOpType.mult)
                        nc.vector.tensor_tensor(
                            tmp[:].rearrange("p (b q) -> p b q", b=2),
                            p_rot[:].rearrange("p (b q) -> p b q", b=2),
                            sin_s[:, None, :].to_broadcast((128, 2, 256)),
                            mybir.AluOpType.mult)
                        nc.vector.tensor_tensor(
                            sb[:, h, :], sb[:, h, :], tmp[:],
                            mybir.AluOpType.add)

                # ---- proj2 for v (token-major), contracts xr tiles {3,4} ----
                vsb = vp_.tile([128, 4, 768], bf16, tag="vsb", name="vsb")
                for mt in range(4):
                    for nch in range(2):
                        vps = psum()
                        for kt in range(2):
                            nc.tensor.matmul(
                                vps[:, 0:384],
                                xr[:, 3 + kt, mt * 128:(mt + 1) * 128],
                                vBp_s[:, kt, nch * 384:(nch + 1) * 384],
                                start=(kt == 0), stop=(kt == 1))
                        nc.scalar.copy(vsb[:, mt, nch * 384:(nch + 1) * 384],
                                       vps[:, 0:384])

                # ---- attention (per batch item) ----
                # E layout per (b, h): [kt0q0 | kt0q1 | kt1q1], each 128 wide.
                aosb = aop.tile([128, 6, 512], bf16, tag="aosb", name="aosb")
                for b in range(2):
                    if prev is not None:
                        emit_outproj(prev[0], prev[1], range(3 * b, 3 * b + 3))
                    i_bD = bp.tile([128, 1536], f32, tag="ibD", name="ibD")
                    Es = []
                    for h in range(6):
                        base = h * 512 + 0  # qsb is [128, 6, 512]
                        qh = qsb[:, h, b * 256:(b + 1) * 256]
                        kh = ksb[:, h, b * 256:(b + 1) * 256]
                        sp = spp.tile([128, 512], f32, tag="sp", name="sp")
                        nc.tensor.matmul(sp[:, 0:256], kh[:, 0:128],
                                         qh[:, 0:256], start=True, stop=True)
                        nc.tensor.matmul(sp[:, 256:384], kh[:, 128:256],
                                         qh[:, 128:256], start=True, stop=True)
                        E = ep.tile([128, 384], bf16, tag="E", name="E")
                        nc.scalar.activation(
                            E[:], sp[:, 0:384],
                            mybir.ActivationFunctionType.Exp, scale=SCALE)
                        # one mask multiply: [tril | ones | tril]
                        nc.vector.tensor_tensor(
                            E[:], E[:], mask_s[:],
                            mybir.AluOpType.mult)
                        Es.append(E)
                    # denominators: all-ones stationary replicates the key-sum
                    # across all 128 partitions (broadcast comes for free)
                    dslots = []
                    for h in range(6):
                        E = Es[h]
                        s, c = h // 2, (h % 2) * 256
                        if h % 2 == 0:
                            dslots.append(psum())
                        dps = dslots[s]
                        nc.tensor.matmul(dps[:, c:c + 256], ones_s[:],
                                         E[:, 0:256], start=True, stop=False,
                                         skip_group_check=True)
                        nc.tensor.matmul(dps[:, c + 128:c + 256], ones_s[:],
                                         E[:, 256:384], start=False, stop=True,
                                         skip_group_check=True)
                        if h % 2 == 1:
                            sc = slice(s * 512, (s + 1) * 512)
                            nc.vector.reciprocal_approx_fast(
                                i_bD[:, sc], dps[:, :])
                    # attention output (feature-major), normalized straight
                    # out of PSUM
                    for h in range(6):
                        E = Es[h]
                        o2 = psum()
                        v0 = vsb[:, b * 2 + 0, h * 128:(h + 1) * 128]
                        v1 = vsb[:, b * 2 + 1, h * 128:(h + 1) * 128]
                        nc.tensor.matmul(o2[:, 0:256], v0, E[:, 0:256],
                                         start=True, stop=False,
                                         skip_group_check=True)
                        nc.tensor.matmul(o2[:, 128:256], v1, E[:, 256:384],
                                         start=False, stop=True,
                                         skip_group_check=True)
                        nc.vector.tensor_tensor(
                            aosb[:, h, b * 256:(b + 1) * 256],
                            o2[:, 0:256],
                            i_bD[:, h * 256:(h + 1) * 256],
                            mybir.AluOpType.mult)
                    if prx == n_pairs - 1:
                        emit_outproj(aosb, prx, range(6), half=b)
                prev = (aosb, prx)

    nc.compile()
    return nc


def _rope_tables():
    inv = 1.0 / (10000.0 ** (np.arange(0, HD, 2, dtype=np.float32) / HD))
    t = np.arange(T, dtype=np.float32)
    freqs = np.outer(t, inv)                      # [T, 64]
    emb = np.concatenate([freqs, freqs], axis=-1)  # [T, 128]
    return np.cos(emb).astype(np.float32), np.sin(emb).astype(np.float32)


def _prep_shared(qA, qB, kA, kB, vA, vB, o_w):
    """Host-side weight/constant layouts (shared by all cores)."""
    def a_r(A):  # [768,192] -> [6,128,192]
        return A.reshape(6, 128, RANK)

    qA_r, kA_r, vA_r = a_r(qA), a_r(kA), a_r(vA)
    Ap = np.zeros((6, 128, 640), np.float32)
    Ap[:, :, 0:128] = qA_r[:, :, 0:128]
    Ap[:, :, 128:192] = qA_r[:, :, 128:192]
    Ap[:, :, 192:256] = kA_r[:, :, 0:64]
    Ap[:, :, 256:384] = kA_r[:, :, 64:192]
    Ap[:, :, 384:512] = vA_r[:, :, 0:128]
    Ap[:, :, 512:576] = vA_r[:, :, 128:192]

    qBp = np.zeros((2, 128, D), np.float32)
    qBp[0] = qB[0:128]
    qBp[1, 0:64] = qB[128:192]

    kBp = np.zeros((2, 128, D), np.float32)
    kBp[0, 64:128] = kB[0:64]
    kBp[1] = kB[64:192]

    vBp = np.zeros((2, 128, D), np.float32)
    vBp[0] = vB[0:128]
    vBp[1, 0:64] = vB[128:192]

    # rotate-half permutation (as matmul lhsT): out[m] = sum_k P[k,m] q[k]
    P = np.zeros((128, 128), np.float32)
    for m in range(64):
        P[m + 64, m] = -1.0
        P[m, m + 64] = 1.0

    cos, sin = _rope_tables()
    cosT = np.ascontiguousarray(cos.T)  # [128, 256]
    sinT = np.ascontiguousarray(sin.T)

    p = np.arange(128)[:, None]
    j = np.arange(128)[None, :]
    tril = (p <= j).astype(np.float32)  # keys (partitions) <= queries (cols)
    mask = np.concatenate(
        [tril, np.ones((128, 128), np.float32), tril], axis=1)  # [128, 384]

    return {
        "Ap_l": _to_bf16(np.ascontiguousarray(Ap)),
        "qBp_l": _to_bf16(qBp), "kBp_l": _to_bf16(kBp), "vBp_l": _to_bf16(vBp),
        "ow_l": _to_bf16(np.ascontiguousarray(o_w.reshape(6, 128, D))),
        "P_l": _to_bf16(P),
        "cosT": _to_bf16(cosT), "sinT": _to_bf16(sinT),
        "mask_l": _to_bf16(mask),
        "ones_l": _to_bf16(np.ones((128, 128), np.float32)),
    }


def x_to_xT(xc):
    """[b, T, D] -> [6, 128, b*T] feature-major, batch-major tokens."""
    nb = xc.shape[0]
    return np.ascontiguousarray(
        _to_bf16(xc).reshape(nb, T, 6, 128).transpose(2, 3, 0, 1).reshape(6, 128, nb * T))


def outT_to_out(oT, nb):
    return np.ascontiguousarray(
        oT.reshape(6, 128, nb, T).transpose(2, 3, 0, 1).reshape(nb, T, D))


def kernel(x, qA, qB, kA, kB, vA, vB, o_w):
    from concourse import bass_utils

    if "nc" not in _CACHE:
        _CACHE["nc"] = build_program(N_PAIRS)
    nc = _CACHE["nc"]

    shared = _prep_shared(
        np.asarray(qA, np.float32), np.asarray(qB, np.float32),
        np.asarray(kA, np.float32), np.asarray(kB, np.float32),
        np.asarray(vA, np.float32), np.asarray(vB, np.float32),
        np.asarray(o_w, np.float32))
    x = np.asarray(x, np.float32)

    in_maps = []
    for c in range(N_CORES):
        m = dict(shared)
        m["xT"] = x_to_xT(x[c * B_LOC:(c + 1) * B_LOC])
        in_maps.append(m)

    res = bass_utils.run_bass_kernel_spmd(
        nc, in_maps, core_ids=list(range(N_CORES)))
    out = np.empty((B, T, D), np.float32)
    for c in range(N_CORES):
        out[c * B_LOC:(c + 1) * B_LOC] = outT_to_out(
            res.results[c]["outT"], B_LOC)
    return out
